# revision 13
# baseline (speedup 1.0000x reference)
"""Trainium2 Bass kernel for nn_ComplexTransformer (complex transformer,
DEPTH=2, B=2, N=1024, DIM=512, HEADS=8, DH=64, FF=2048).

Sharding: 2048 tokens (B*N) split 8 ways, 256 tokens/core; cores 0-3 own
batch 0, cores 4-7 batch 1.  Token-parallel everywhere except attention,
which AllGathers post-rotary K and V (fp8 payload, one merged collective
per head-half per layer -> 2 collectives/layer, pipelined against
attention on the first head-half).

All projection matmuls run fp8e4m3 with DoubleRow perf mode: complex
matmuls are "term-paired" -- the two real terms of each output plane form
the DoubleRow pair, with the negation folded into host-prepared weights:
  out_r = Wr.h_r + (-Wi).h_i ;  out_i = Wi.h_r + Wr.h_i
Weights are scaled x16 (x4 for W1) on the host for fp8 range; the inverse
scales fold into exp scale / residual adds / the modrelu sqrt.

Attention: scores fp8 (plain matmul), exp on ACT into fp8 "at" tiles
([128,4,256] batched over 4 key-blocks), AV as DoubleRow over key-block
pairs with an appended ones-column for the softmax denominator.
"""

import os
import sys

_jp = os.environ.get("JAX_PLATFORMS")
if _jp is not None and _jp.strip() and "axon" not in _jp:
    os.environ["JAX_PLATFORMS"] = ""

for _p in ("/opt/trn_rl_repo/concourse", "/opt/trn_rl_repo"):
    if _p not in sys.path:
        sys.path.insert(0, _p)

import ml_dtypes
import numpy as np

import concourse.bass as bass
import concourse.bacc as bacc
import concourse.mybir as mybir
import concourse.tile as tile
from concourse.bass_utils import run_bass_kernel_spmd
from concourse.masks import make_identity

F32 = mybir.dt.float32
BF16 = mybir.dt.bfloat16
FP8 = mybir.dt.float8e4
AF = mybir.ActivationFunctionType
ALU = mybir.AluOpType
DR = mybir.MatmulPerfMode.DoubleRow

# model dims
L = 2
B = 2
N = 1024
DIM = 512
HEADS = 8
DH = 64
INNER = 512
FF = 2048
EPS = 1e-6
SCALE = DH ** -0.5

# sharding dims
NCORES = 8
TOK = 256      # tokens per core
TB = 2         # 128-token blocks per core
KB = 4         # 128-dim blocks of DIM/INNER
JB = 8         # 128-token key blocks per batch (N/128)
OBF = 16       # 128-dim blocks of FF
HP = 4         # head pairs

WS = 16.0      # fp8 weight scale (wq/wk/wv/wo/w2)
WS1 = 16.0     # fp8 weight scale for W1
LAM = 32.0     # fp8 scale of modrelu output ap = a*mag
VSLOT = 80     # per-head slot width in v_all (64 v + 1 ones + 15 pad)

# combo table: (k plane, q plane, v plane, exp-scale sign)
# rr: qr*kr vr ; ri: qr*(-ki) vr ; ir: qi*kr vi ; ii: qi*(-ki) vi
COMBOS = [(0, 0, 0, 1.0), (1, 0, 0, -1.0), (0, 1, 1, 1.0), (1, 1, 1, -1.0)]


def build_nc(taps=False, unit_gamma=False, zero_mb=False):
    nc = bacc.Bacc("TRN2", target_bir_lowering=False, num_devices=NCORES)

    # ---- I/O ----
    x_in = nc.dram_tensor("x", [2, TB, 128, DIM], F32, kind="ExternalInput")
    # stationary term-paired weights [L, (q|k), outpl, ob, 128k, KB, 2, 128m]
    wqk = nc.dram_tensor("wqk", [L, 2, 2, 4, 128, KB, 2, 128], FP8,
                         kind="ExternalInput")
    w1 = nc.dram_tensor("w1", [L, 2, 4, 128, 4, KB, 2, 128], BF16,
                        kind="ExternalInput")
    # moving term-paired weights [L, outpl, 128k, kc, 2, OUT]
    wv = nc.dram_tensor("wv", [L, 2, 128, KB, 2, INNER], FP8,
                        kind="ExternalInput")
    wo = nc.dram_tensor("wo", [L, 2, 128, HP, 2, DIM], FP8,
                        kind="ExternalInput")
    w2 = nc.dram_tensor("w2", [L, 2, OBF, 128, 2, DIM], BF16,
                        kind="ExternalInput")
    b1c = nc.dram_tensor("b1c", [L, 2, 128, OBF], F32, kind="ExternalInput")
    b2b = nc.dram_tensor("b2b", [L, 2, 128, DIM], F32, kind="ExternalInput")
    g_at = nc.dram_tensor("g_at", [L, 2, 128, DIM], F32, kind="ExternalInput")
    g_ff = nc.dram_tensor("g_ff", [L, 2, 128, DIM], F32, kind="ExternalInput")
    g_fin = nc.dram_tensor("g_fin", [2, 128, DIM], F32, kind="ExternalInput")
    rotc = nc.dram_tensor("rotc", [128, TOK], F32, kind="ExternalInput")
    rots = nc.dram_tensor("rots", [128, TOK], F32, kind="ExternalInput")
    mbias = nc.dram_tensor("mbias", [128, L], F32, kind="ExternalInput")
    out_d = nc.dram_tensor("out", [2, TB, 128, DIM], F32, kind="ExternalOutput")

    tap_d = {}
    if taps:
        for name, shape, dt_ in [
            ("hT0", [128, 2, KB, TOK], FP8),
            ("qT0", [128, 2, HP, TOK], FP8),
            ("kT0", [128, KB, 2, 1024], FP8),
            ("vA0", [128, 2, JB, 2, 4 * VSLOT], FP8),
            ("oT0", [128, 2, HP, TOK], FP8),
            ("xc1", [2, TB, 128, DIM], F32),
            ("xc2", [2, TB, 128, DIM], F32),
        ]:
            tap_d[name] = nc.dram_tensor(name, shape, dt_, kind="ExternalOutput")

    replica_groups = [[0, 1, 2, 3], [4, 5, 6, 7]]
    uid = [0]

    def un(s):
        uid[0] += 1
        return f"{s}{uid[0]}"

    from contextlib import ExitStack

    with tile.TileContext(nc) as tc, ExitStack() as _es:
        def pool(name, bufs, space="SBUF"):
            return _es.enter_context(
                tc.tile_pool(name=name, bufs=bufs, space=space))

        consts = pool("consts", 1)
        xcp = pool("xcp", 1)
        hTp = pool("hTp", 1)
        qTp = pool("qTp", 1)
        kfullp = pool("kfull", 1)
        vallp = pool("vall", 1)
        oTp = pool("oTp", 1)
        ap2p = pool("ap2p", 1)
        gbp = pool("gb", 2)
        wsm = pool("wsm", 6)       # small lhsT weights
        wmv = pool("wmv", 4)       # wv/wo moving tiles
        wbig = pool("wbig", 2)     # w1 big tiles
        ntmp = pool("ntmp", 1)
        smallp = pool("small", 4)
        atp = pool("atp", 4)
        ocp = pool("ocp", 4)
        ffp = pool("ffp", 2)
        dramp = pool("dram", 2, space="DRAM")
        if True:
            ident = consts.tile([128, 128], BF16)
            make_identity(nc, ident)
            rc_t = consts.tile([128, TOK], F32)
            rs_t = consts.tile([128, TOK], F32)
            nc.sync.dma_start(out=rc_t[:], in_=rotc[:])
            nc.sync.dma_start(out=rs_t[:], in_=rots[:])
            mb_t = consts.tile([128, L], F32)
            nc.sync.dma_start(out=mb_t[:], in_=mbias[:])
            eps_t = consts.tile([128, 1], F32)
            nc.vector.memset(eps_t[:], EPS)
            tiny_t = consts.tile([128, 1], F32)
            nc.vector.memset(tiny_t[:], 1e-30)

            # residual stream
            xc = [xcp.tile([128, TB, DIM], F32, name=f"xc{p}") for p in range(2)]
            for p in range(2):
                for tb in range(TB):
                    nc.sync.dma_start(out=xc[p][:, tb, :], in_=x_in[p, tb])

            # gathered K (post-rotary) and V for the whole batch
            k_full = kfullp.tile([128, KB, 2, 1024], FP8, name="kfull")
            v_all = vallp.tile([128, 2, JB, 2, 4 * VSLOT], FP8, name="vall")
            va6 = v_all[:].rearrange("q h j p (s c) -> q h j p s c", s=4)

            def norm_natural(g_dram, h_nat, dtype):
                """rmsnorm(xc)*gamma in natural layout into h_nat tiles."""
                if not unit_gamma:
                    g0 = ntmp.tile([128, DIM], F32, name="g0", bufs=1)
                    g1 = ntmp.tile([128, DIM], F32, name="g1", bufs=1)
                    nc.sync.dma_start(out=g0[:], in_=g_dram[0])
                    nc.sync.dma_start(out=g1[:], in_=g_dram[1])
                for tb in range(TB):
                    sq = ntmp.tile([128, DIM], F32, name="sq")
                    ss0 = ntmp.tile([128, 1], F32, name="ss0")
                    ss1 = ntmp.tile([128, 1], F32, name="ss1")
                    nc.scalar.activation(sq[:], xc[0][:, tb, :], AF.Square,
                                         accum_out=ss0[:])
                    sq2 = ntmp.tile([128, DIM], F32, name="sq2")
                    nc.scalar.activation(sq2[:], xc[1][:, tb, :], AF.Square,
                                         accum_out=ss1[:])
                    s_all = ntmp.tile([128, 1], F32, name="s_all")
                    nc.vector.tensor_add(s_all[:], ss0[:], ss1[:])
                    rms = ntmp.tile([128, 1], F32, name="rms")
                    nc.scalar.activation(rms[:], s_all[:], AF.Sqrt,
                                         scale=1.0 / DIM, bias=eps_t[:])
                    rstd = ntmp.tile([128, 1], F32, name="rstd")
                    nc.vector.reciprocal(rstd[:], rms[:])
                    if unit_gamma:
                        nc.vector.tensor_scalar_mul(h_nat[0][:, tb, :],
                                                    xc[0][:, tb, :], rstd[:])
                        nc.vector.tensor_scalar_mul(h_nat[1][:, tb, :],
                                                    xc[1][:, tb, :], rstd[:])
                        continue
                    # h_r = (xr*gr - xi*gi)*rstd ; h_i = (xr*gi + xi*gr)*rstd
                    t1 = ntmp.tile([128, DIM], F32, name="t1")
                    t2 = ntmp.tile([128, DIM], F32, name="t2")
                    nc.vector.tensor_mul(t1[:], xc[0][:, tb, :], g0[:])
                    nc.vector.scalar_tensor_tensor(
                        t2[:], xc[1][:, tb, :], -1.0, g1[:],
                        op0=ALU.mult, op1=ALU.mult)
                    u = ntmp.tile([128, DIM], F32, name="u")
                    nc.vector.tensor_add(u[:], t1[:], t2[:])
                    nc.vector.tensor_scalar_mul(h_nat[0][:, tb, :], u[:], rstd[:])
                    t3 = ntmp.tile([128, DIM], F32, name="t1")
                    t4 = ntmp.tile([128, DIM], F32, name="t2")
                    nc.vector.tensor_mul(t3[:], xc[0][:, tb, :], g1[:])
                    nc.vector.tensor_mul(t4[:], xc[1][:, tb, :], g0[:])
                    u2 = ntmp.tile([128, DIM], F32, name="u")
                    nc.vector.tensor_add(u2[:], t3[:], t4[:])
                    nc.vector.tensor_scalar_mul(h_nat[1][:, tb, :], u2[:], rstd[:])

            def rmsnorm_T_fp8(g_dram, dtype=FP8, tap=None):
                """rmsnorm(xc), transposed into hT [128, 2pl, KB, TOK]."""
                hn0 = ntmp.tile([128, TB, DIM], BF16, name="hn0", bufs=1)
                hn1 = ntmp.tile([128, TB, DIM], BF16, name="hn1", bufs=1)
                h_nat = [hn0, hn1]
                norm_natural(g_dram, h_nat, BF16)
                hT = hTp.tile([128, 2, KB, TOK], dtype, name="hT")
                with tc.tile_pool(name=un("pst"), bufs=2, space="PSUM") as pst:
                    for p in range(2):
                        for tb in range(TB):
                            pt = pst.tile([128, KB, 128], BF16, name="pt")
                            for kb in range(KB):
                                nc.tensor.transpose(
                                    pt[:, kb, :],
                                    h_nat[p][:, tb, kb * 128:(kb + 1) * 128],
                                    ident[:])
                            nc.vector.tensor_copy(
                                hT[:, p, :, tb * 128:(tb + 1) * 128], pt[:])
                if tap is not None:
                    nc.sync.dma_start(out=tap, in_=hT[:])
                return hT

            def attn_block(l, dotap):
                hT = rmsnorm_T_fp8(g_at[l],
                                   tap=tap_d["hT0"][:] if dotap else None)

                qT = qTp.tile([128, 2, HP, TOK], FP8, name="qT")
                kT = qTp.tile([128, KB, 2, TOK], FP8, name="kT")
                v_loc = qTp.tile([128, 2, TB, 2, 4 * VSLOT], FP8, name="vloc")
                vl6 = v_loc[:].rearrange("q h t p (s c) -> q h t p s c", s=4)
                nc.vector.memset(vl6[:, :, :, :, :, 64:VSLOT], 0.0)
                nc.vector.memset(vl6[:, :, :, :, :, 64:65], 1.0)

                def proj_rot(dst, qk, psp, kb_major=False):
                    """q/k projection + rotary for all 4 obs into dst."""
                    for ob in range(KB):
                        ps = []
                        for opl in range(2):
                            wt = wsm.tile([128, KB, 2, 128], FP8, name="wt")
                            nc.sync.dma_start(out=wt[:], in_=wqk[l, qk, opl, ob])
                            p_ = psp.tile([128, TOK], F32, name="pq")
                            for kb in range(KB):
                                nc.tensor.matmul(
                                    p_[:], wt[:, kb, :, :], hT[:, :, kb, :],
                                    start=(kb == 0), stop=(kb == KB - 1),
                                    perf_mode=DR)
                            ps.append(p_)
                        # rotary: out_r = pr*c - pi*s; out_i = pr*s + pi*c
                        t1 = smallp.tile([128, TOK], F32, name="r1", bufs=2)
                        t2 = smallp.tile([128, TOK], F32, name="r2", bufs=2)
                        nc.vector.tensor_mul(t1[:], ps[0][:], rc_t[:])
                        nc.vector.scalar_tensor_tensor(
                            t2[:], ps[1][:], -1.0, rs_t[:],
                            op0=ALU.mult, op1=ALU.mult)
                        d0 = dst[:, ob, 0, :] if kb_major else dst[:, 0, ob, :]
                        d1 = dst[:, ob, 1, :] if kb_major else dst[:, 1, ob, :]
                        nc.vector.tensor_add(d0, t1[:], t2[:])
                        t3 = smallp.tile([128, TOK], F32, name="r3", bufs=2)
                        t4 = smallp.tile([128, TOK], F32, name="r4", bufs=2)
                        nc.vector.tensor_mul(t3[:], ps[0][:], rs_t[:])
                        nc.vector.tensor_mul(t4[:], ps[1][:], rc_t[:])
                        nc.vector.tensor_add(d1, t3[:], t4[:])

                def v_proj_half(half, psp):
                    """v projection for inner cols half*256:(half+1)*256."""
                    cs = slice(half * 256, (half + 1) * 256)
                    for opl in range(2):
                        wvt = wmv.tile([128, KB, 2, 256], FP8, name="wvt")
                        nc.sync.dma_start(out=wvt[:], in_=wv[l, opl, :, :, :, cs])
                        for tb in range(TB):
                            p_ = psp.tile([128, 256], F32, name="pv")
                            for kb in range(KB):
                                nc.tensor.matmul(
                                    p_[:],
                                    hT[:, :, kb, tb * 128:(tb + 1) * 128],
                                    wvt[:, kb, :, :],
                                    start=(kb == 0), stop=(kb == KB - 1),
                                    perf_mode=DR)
                            pv4 = p_[:].rearrange("q (s c) -> q s c", s=4)
                            nc.vector.tensor_copy(
                                vl6[:, half, tb, opl, :, 0:64], pv4)

                gath = []
                with tc.tile_pool(name=un("psp"), bufs=4, space="PSUM") as psp:
                    # k proj (all obs; halves kb 0-1 / 2-3), v halves, bounce
                    proj_rot(kT, 1, psp, kb_major=True)
                    for half in range(2):
                        v_proj_half(half, psp)
                        bounce = dramp.tile([128, 2304], FP8,
                                            name=f"bounce{half}")
                        gt = dramp.tile([4, 128, 2304], FP8, name=f"gath{half}")
                        nc.sync.dma_start(
                            out=bounce[:, 0:1024],
                            in_=kT[:, half * 2:half * 2 + 2, :, :])
                        nc.sync.dma_start(
                            out=bounce[:, 1024:2304],
                            in_=v_loc[:, half])
                        nc.gpsimd.collective_compute(
                            "AllGather", ALU.bypass,
                            replica_groups=replica_groups,
                            ins=[bounce[:].opt()], outs=[gt[:].opt()])
                        gath.append(gt)
                    # q projection overlaps the collectives
                    proj_rot(qT, 0, psp)

                if dotap:
                    nc.sync.dma_start(out=tap_d["qT0"][:], in_=qT[:])

                def land_half(half):
                    gt = gath[half]
                    for s_ in range(4):
                        gk = gt[s_, :, 0:1024].rearrange(
                            "q (k p t) -> q k p t", k=2, p=2)
                        nc.sync.dma_start(
                            out=k_full[:, half * 2:half * 2 + 2, :,
                                       s_ * 256:(s_ + 1) * 256],
                            in_=gk)
                        nc.sync.dma_start(
                            out=v_all[:, half, s_ * 2:s_ * 2 + 2, :, :],
                            in_=gt[s_, :, 1024:2304])

                oT = oTp.tile([128, 2, HP, TOK], FP8, name="oT")
                with (
                    tc.tile_pool(name=un("pss"), bufs=2, space="PSUM") as pss,
                    tc.tile_pool(name=un("psa"), bufs=2, space="PSUM") as psa,
                ):
                    for hp in range(HP):
                        if hp % 2 == 0:
                            land_half(hp // 2)
                        for h01 in range(2):
                            pr = slice(64 * h01, 64 * h01 + 64)
                            slot = 2 * hp + h01
                            av = psa.tile([65, 4, TOK], F32, name="av")
                            for ci, (kpl, qpl, vpl, sgn) in enumerate(COMBOS):
                                for hj in range(2):
                                    sp = pss.tile([128, 4, TOK], F32, name="sp")
                                    for q4 in range(4):
                                        jb = hj * 4 + q4
                                        nc.tensor.matmul(
                                            sp[:, q4, :],
                                            k_full[pr, hp, kpl,
                                                   jb * 128:(jb + 1) * 128],
                                            qT[pr, qpl, hp, :],
                                            start=True, stop=True)
                                    at = atp.tile([128, 4, TOK], FP8, name="at")
                                    nc.scalar.activation(
                                        at[:], sp[:], AF.Exp,
                                        scale=float(sgn * SCALE / (WS * WS)))
                                    for jq in range(2):
                                        jj = hj * 2 + jq
                                        nc.tensor.matmul(
                                            av[:, ci, :],
                                            va6[:, hp // 2,
                                                2 * jj:2 * jj + 2, vpl,
                                                2 * (hp % 2) + h01, 0:65],
                                            at[:, 2 * jq:2 * jq + 2, :],
                                            start=(jj == 0), stop=(jj == 3),
                                            perf_mode=DR)
                            # combine: oc = av/denom ; o_r = oc0-oc3,
                            # o_i = oc1+oc2
                            rec = smallp.tile([1, 4, TOK], F32, name="rec", bufs=2)
                            nc.vector.reciprocal(rec[:], av[64:65, :, :])
                            bc = smallp.tile([64, 4, TOK], F32, name="bc", bufs=2)
                            nc.gpsimd.partition_broadcast(bc[:], rec[:])
                            oc = ocp.tile([64, 4, TOK], F32, name="oc", bufs=2)
                            nc.vector.tensor_mul(oc[:], av[0:64, :, :], bc[:])
                            if h01 == 0:
                                d_r = oT[0:64, 0, hp, :]
                                d_i = oT[0:64, 1, hp, :]
                            else:
                                st = ocp.tile([64, 2, TOK], FP8, name="st", bufs=2)
                                d_r = st[:, 0, :]
                                d_i = st[:, 1, :]
                            nc.vector.tensor_sub(d_r, oc[:, 0, :], oc[:, 3, :])
                            nc.vector.tensor_add(d_i, oc[:, 1, :], oc[:, 2, :])
                            if h01 == 1:
                                nc.sync.dma_start(out=oT[64:128, :, hp, :],
                                                  in_=st[:])

                if dotap:
                    nc.sync.dma_start(out=tap_d["kT0"][:], in_=k_full[:])
                    nc.sync.dma_start(out=tap_d["vA0"][:], in_=v_all[:])
                    nc.sync.dma_start(out=tap_d["oT0"][:], in_=oT[:])

                # ---- Wo projection + residual (psum = 256x true) ----
                with tc.tile_pool(name=un("psw"), bufs=4, space="PSUM") as psw:
                    for opl in range(2):
                        wot = wmv.tile([128, HP, 2, DIM], FP8, name="wot", bufs=2)
                        nc.sync.dma_start(out=wot[:], in_=wo[l, opl])
                        for tb in range(TB):
                            p_ = psw.tile([128, DIM], F32, name="po")
                            for hp in range(HP):
                                nc.tensor.matmul(
                                    p_[:],
                                    oT[:, :, hp, tb * 128:(tb + 1) * 128],
                                    wot[:, hp, :, :],
                                    start=(hp == 0), stop=(hp == HP - 1),
                                    perf_mode=DR)
                            nc.vector.scalar_tensor_tensor(
                                xc[opl][:, tb, :], p_[:], 1.0 / (WS * WS),
                                xc[opl][:, tb, :],
                                op0=ALU.mult, op1=ALU.add)

            def ff_block(l, dotap):
                hT = rmsnorm_T_fp8(g_ff[l], dtype=BF16)
                b1t = gbp.tile([128, 2, OBF], F32, name="b1t")
                nc.sync.dma_start(out=b1t[:, 0, :], in_=b1c[l, 0])
                nc.sync.dma_start(out=b1t[:, 1, :], in_=b1c[l, 1])
                b2t = gbp.tile([128, 2, DIM], F32, name="b2t")
                nc.sync.dma_start(out=b2t[:, 0, :], in_=b2b[l, 0])
                nc.sync.dma_start(out=b2t[:, 1, :], in_=b2b[l, 1])

                ap2 = ap2p.tile([128, 2, OBF, TOK], BF16, name="ap2")
                with tc.tile_pool(name=un("psa"), bufs=4, space="PSUM") as psa:
                    for ob in range(OBF):
                        if ob % 4 == 0:
                            w1q = [wbig.tile([128, 4, KB, 2, 128], BF16,
                                             name=f"w1q{opl}", bufs=2)
                                   for opl in range(2)]
                            for opl in range(2):
                                nc.sync.dma_start(out=w1q[opl][:],
                                                  in_=w1[l, opl, ob // 4])
                        pA = []
                        for opl in range(2):
                            p_ = psa.tile([128, TOK], F32, name="pA")
                            i = 0
                            for kb in range(KB):
                                for j in range(2):
                                    nc.tensor.matmul(
                                        p_[:], w1q[opl][:, ob % 4, kb, j, :],
                                        hT[:, j, kb, :],
                                        start=(i == 0), stop=(i == 2 * KB - 1))
                                    i += 1
                            pA.append(p_)
                        # modrelu (unscaled bf16 flow)
                        a_r = ffp.tile([128, TOK], BF16, name="a_r")
                        a_i = ffp.tile([128, TOK], BF16, name="a_i")
                        nc.vector.tensor_scalar_add(a_r[:], pA[0][:],
                                                    b1t[:, 0, ob:ob + 1])
                        nc.vector.tensor_scalar_add(a_i[:], pA[1][:],
                                                    b1t[:, 1, ob:ob + 1])
                        sq_r = ffp.tile([128, TOK], BF16, name="sq_r")
                        sq_i = ffp.tile([128, TOK], BF16, name="sq_i")
                        nc.vector.tensor_mul(sq_r[:], a_r[:], a_r[:])
                        nc.vector.tensor_mul(sq_i[:], a_i[:], a_i[:])
                        m2 = ffp.tile([128, TOK], BF16, name="m2")
                        nc.vector.tensor_add(m2[:], sq_r[:], sq_i[:])
                        fac = ffp.tile([128, TOK], BF16, name="fac")
                        if zero_mb:
                            nc.scalar.activation(fac[:], m2[:], AF.Sqrt)
                        else:
                            mag = ffp.tile([128, TOK], F32, name="mag")
                            nc.scalar.activation(mag[:], m2[:], AF.Sqrt,
                                                 bias=tiny_t[:])
                            rel = ffp.tile([128, TOK], F32, name="rel")
                            nc.scalar.activation(rel[:], mag[:], AF.Relu,
                                                 bias=mb_t[:, l:l + 1])
                            rel2 = ffp.tile([128, TOK], F32, name="rel2")
                            nc.vector.tensor_mul(rel2[:], rel[:], rel[:])
                            rmag = ffp.tile([128, TOK], F32, name="rmag")
                            nc.vector.reciprocal(rmag[:], mag[:])
                            nc.vector.tensor_mul(fac[:], rel2[:], rmag[:])
                        nc.vector.tensor_mul(ap2[:, 0, ob, :], a_r[:], fac[:])
                        nc.vector.tensor_mul(ap2[:, 1, ob, :], a_i[:], fac[:])

                # W2 (ob-outer accumulation, bf16) + bias + residual
                with tc.tile_pool(name=un("ps2"), bufs=4, space="PSUM") as ps2:
                    w2ps = [[ps2.tile([128, DIM], F32, name="p2", bufs=4)
                             for _ in range(TB)] for _ in range(2)]
                    for ob in range(OBF):
                        w2t = wmv.tile([128, 2, 2, DIM], BF16, name="w2t",
                                       bufs=4)
                        nc.sync.dma_start(
                            out=w2t[:], in_=w2[l, :, ob].rearrange(
                                "o q j d -> q o j d"))
                        for opl in range(2):
                            for tb in range(TB):
                                i = 0
                                for j in range(2):
                                    nc.tensor.matmul(
                                        w2ps[opl][tb][:],
                                        ap2[:, j, ob, tb * 128:(tb + 1) * 128],
                                        w2t[:, opl, j, :],
                                        start=(ob == 0 and i == 0),
                                        stop=(ob == OBF - 1 and i == 1))
                                    i += 1
                    for opl in range(2):
                        for tb in range(TB):
                            nc.vector.tensor_add(xc[opl][:, tb, :],
                                                 xc[opl][:, tb, :],
                                                 w2ps[opl][tb][:])
                            nc.vector.tensor_add(xc[opl][:, tb, :],
                                                 xc[opl][:, tb, :],
                                                 b2t[:, opl, :])

            for l in range(L):
                attn_block(l, taps and l == 0)
                if taps and l == 0:
                    for p in range(2):
                        for tb in range(TB):
                            nc.sync.dma_start(out=tap_d["xc1"][p, tb],
                                              in_=xc[p][:, tb, :])
                ff_block(l, taps and l == 0)
                if taps and l == 0:
                    for p in range(2):
                        for tb in range(TB):
                            nc.sync.dma_start(out=tap_d["xc2"][p, tb],
                                              in_=xc[p][:, tb, :])

            # ---- final norm + output ----
            fo0 = ntmp.tile([128, TB, DIM], F32, name="fn0", bufs=1)
            fo1 = ntmp.tile([128, TB, DIM], F32, name="fn1", bufs=1)
            norm_natural(g_fin, [fo0, fo1], F32)
            for p_out, t in ((0, fo0), (1, fo1)):
                for tb in range(TB):
                    nc.sync.dma_start(out=out_d[p_out, tb], in_=t[:, tb, :])

    nc.compile()
    return nc


# ---------------------------------------------------------------------------
# host side: shard, run, unshard
# ---------------------------------------------------------------------------

FP8NP = ml_dtypes.float8_e4m3


def _prep_shared(Wq, Wkv, Wo, W1, b1, W2, b2, gamma_attn, gamma_ff, mod_bias,
                 gamma_final):
    """Host-side marshalling of the weight tensors (identical on all cores)."""
    def lp(w):  # [L, ..., 2] -> [L, 2, ...]
        return np.moveaxis(np.moveaxis(w, -1, 0), 0, 1)

    sh = {}
    wq_p = lp(Wq)                       # [L, 2, DIM, INNER]
    wk_p = lp(Wkv[:, :, :INNER, :])
    wv_p = lp(Wkv[:, :, INNER:, :])
    wo_p = lp(Wo)                       # [L, 2, INNER, DIM]
    w1_p = lp(W1)                       # [L, 2, DIM, FF]
    w2_p = lp(W2)                       # [L, 2, FF, DIM]

    def pair(w_p, scale):
        """[L, 2, K, M] -> term pairs [L, outpl, 2pair, K, M]."""
        Wr = w_p[:, 0] * scale
        Wi = w_p[:, 1] * scale
        return np.stack([np.stack([Wr, -Wi], axis=1),
                         np.stack([Wi, Wr], axis=1)], axis=1)

    def stationary(w_p, nob, scale, k_major=False):
        """-> [L, 2outpl, nob, 128k, KB, 2pair, 128m] (or k-major)."""
        P = pair(w_p, scale)                       # [L, 2, 2, K, M]
        P = P.reshape(L, 2, 2, KB, 128, nob, 128)  # [L,o,pr,kb,k,ob,m]
        if k_major:
            P = P.transpose(0, 1, 4, 5, 3, 2, 6)   # [L,o,k,ob,kb,pr,m]
        else:
            P = P.transpose(0, 1, 5, 4, 3, 2, 6)   # [L,o,ob,k,kb,pr,m]
        return np.ascontiguousarray(P).astype(FP8NP)

    def moving(w_p, scale, kc):
        """-> [L, 2outpl, 128k, kc, 2pair, OUT]."""
        P = pair(w_p, scale)                       # [L, 2, 2, K, OUT]
        P = P.reshape(L, 2, 2, kc, 128, -1)        # [L,o,pr,kc,k,out]
        P = P.transpose(0, 1, 4, 3, 2, 5)          # [L,o,k,kc,pr,out]
        return np.ascontiguousarray(P).astype(FP8NP)

    sh["wqk"] = np.ascontiguousarray(np.stack(
        [stationary(wq_p, 4, WS), stationary(wk_p, 4, WS)], axis=1))
    # w1 bf16: [L, o, q4, 128k, ob4, KB, 2, 128m]
    P = pair(w1_p, 1.0)                            # [L, 2, 2, DIM, FF]
    P = P.reshape(L, 2, 2, KB, 128, 4, 4, 128)     # [L,o,pr,kb,k,q,ob4,m]
    P = P.transpose(0, 1, 5, 4, 6, 3, 2, 7)        # [L,o,q,k,ob4,kb,pr,m]
    sh["w1"] = np.ascontiguousarray(P).astype(ml_dtypes.bfloat16)
    sh["wv"] = moving(wv_p, WS, KB)
    sh["wo"] = moving(wo_p, WS, HP)
    # w2 bf16: [L, o, OBF, 128k, 2, DIM]
    P = pair(w2_p, 1.0)                            # [L, 2, 2, FF, DIM]
    P = P.reshape(L, 2, 2, OBF, 128, DIM)          # [L,o,pr,ob,k,d]
    P = P.transpose(0, 1, 3, 4, 2, 5)              # [L,o,ob,k,pr,d]
    sh["w2"] = np.ascontiguousarray(P).astype(ml_dtypes.bfloat16)

    b1_p = lp(b1)                       # [L, 2, FF]
    sh["b1c"] = np.ascontiguousarray(
        b1_p.reshape(L, 2, OBF, 128).transpose(0, 1, 3, 2))
    b2_p = lp(b2)                       # [L, 2, DIM]
    sh["b2b"] = np.ascontiguousarray(
        np.broadcast_to(b2_p[:, :, None, :], (L, 2, 128, DIM)))
    ga = lp(gamma_attn)                 # [L, 2, DIM]
    sh["g_at"] = np.ascontiguousarray(
        np.broadcast_to(ga[:, :, None, :], (L, 2, 128, DIM)))
    gf = lp(gamma_ff)
    sh["g_ff"] = np.ascontiguousarray(
        np.broadcast_to(gf[:, :, None, :], (L, 2, 128, DIM)))
    gfin = np.moveaxis(gamma_final, -1, 0)      # [2, DIM]
    sh["g_fin"] = np.ascontiguousarray(
        np.broadcast_to(gfin[:, None, :], (2, 128, DIM)))
    sh["mbias"] = np.ascontiguousarray(
        np.broadcast_to(mod_bias[None, :], (128, L)).astype(np.float32))
    return sh


def _rot_tables(core):
    """cos/sin tables [128, TOK] for this core's token positions."""
    inv_freq = 1.0 / (10000.0 ** (np.arange(DH, dtype=np.float64) / DH))
    pos = (core % 4) * TOK + np.arange(TOK, dtype=np.float64)
    dh_idx = np.arange(128) % DH
    freqs = pos[None, :] * inv_freq[dh_idx][:, None]    # [128, TOK]
    return (np.cos(freqs).astype(np.float32),
            np.sin(freqs).astype(np.float32))


_NC_CACHE = {}


def get_nc(taps, unit_gamma, zero_mb):
    key = (taps, unit_gamma, zero_mb)
    if key not in _NC_CACHE:
        _NC_CACHE[key] = build_nc(taps=taps, unit_gamma=unit_gamma,
                                  zero_mb=zero_mb)
    return _NC_CACHE[key]


def make_in_maps(x, gamma_attn, Wq, Wkv, Wo, gamma_ff, W1, b1, mod_bias, W2,
                 b2, gamma_final):
    x = np.asarray(x, dtype=np.float32)
    sh = _prep_shared(np.asarray(Wq, np.float32), np.asarray(Wkv, np.float32),
                      np.asarray(Wo, np.float32), np.asarray(W1, np.float32),
                      np.asarray(b1, np.float32), np.asarray(W2, np.float32),
                      np.asarray(b2, np.float32),
                      np.asarray(gamma_attn, np.float32),
                      np.asarray(gamma_ff, np.float32),
                      np.asarray(mod_bias, np.float32),
                      np.asarray(gamma_final, np.float32))
    xf = x.reshape(B * N, DIM, 2)
    in_maps = []
    for core in range(NCORES):
        tok = xf[core * TOK:(core + 1) * TOK]           # [TOK, DIM, 2]
        xs = np.ascontiguousarray(
            tok.transpose(2, 0, 1).reshape(2, TB, 128, DIM))
        rc, rs = _rot_tables(core)
        m = dict(sh)
        m["x"] = xs
        m["rotc"] = rc
        m["rots"] = rs
        in_maps.append(m)
    return in_maps


def _flags(gamma_attn, gamma_ff, gamma_final, mod_bias):
    def unit(g):
        g = np.asarray(g, np.float32)
        return bool(np.all(g[..., 0] == 1.0) and np.all(g[..., 1] == 0.0))

    unit_gamma = unit(gamma_attn) and unit(gamma_ff) and unit(gamma_final)
    zero_mb = bool(np.all(np.asarray(mod_bias) == 0.0))
    return unit_gamma, zero_mb


def kernel(x, gamma_attn, Wq, Wkv, Wo, gamma_ff, W1, b1, mod_bias, W2, b2,
           gamma_final):
    unit_gamma, zero_mb = _flags(gamma_attn, gamma_ff, gamma_final, mod_bias)
    nc = get_nc(False, unit_gamma, zero_mb)
    in_maps = make_in_maps(x, gamma_attn, Wq, Wkv, Wo, gamma_ff, W1, b1,
                           mod_bias, W2, b2, gamma_final)
    res = run_bass_kernel_spmd(nc, in_maps, core_ids=list(range(NCORES)))
    outs = []
    for core in range(NCORES):
        o = res.results[core]["out"]                    # [2, TB, 128, DIM]
        o = o.reshape(2, TOK, DIM).transpose(1, 2, 0)   # [TOK, DIM, 2]
        outs.append(o)
    full = np.concatenate(outs, axis=0).reshape(B, N, DIM, 2)
    return np.ascontiguousarray(full.astype(np.float32))
